# revision 1
# baseline (speedup 1.0000x reference)
"""Trainium2 Bass kernel for nn_EnsembleTransitionModel.

Sharding: model-parallel (expert-parallel). M=8 ensemble members across 8
NeuronCores; each core runs one full MLP over the whole batch. Inputs are
replicated, per-model weights are sharded.

Device layout: activations are kept feature-major (x^T: [features, batch])
so every weight matrix is used directly as the matmul stationary operand
lhsT=[K_in, M_out] without any transposes. BatchNorm (eval) is folded into
a per-feature affine (scale, bias) applied by one scalar-engine Relu
activation straight out of PSUM. The residual z_last (= x rows 1536:1920)
rides a separate fp32 path so the dominant output term stays full precision.

Matmul dtype modes:
  bf16 — weights/activations bf16: LDWEIGHTS is a separate instruction the
         PE pulls ahead of in-flight matmuls, hiding the weight load.
  f32r — rounded fp32 (full-rate 1 cycle/row) but self-loading matmuls pay
         the ~85-cycle weight load serially.
"""

import os
import sys

import numpy as np

for _p in ("/opt/trn_rl_repo", "/root/.axon_site/_ro/trn_rl_repo"):
    if os.path.isdir(_p) and _p not in sys.path:
        sys.path.insert(0, _p)

M = 8
B = 16384
HIST = 5
L = 384
A = 1
HID = 512
NHL = 2
DIN = L * HIST + A * HIST  # 1925
EPS = 1e-5

DT_MODE = "bf16"  # "bf16" | "f32r"

NCH = 512  # batch columns per chunk (= max fp32 moving dim = 1 PSUM bank)
KT1 = 16  # x^T padded to 2048 rows; device uses k-tiles 0..14 (z part) only:
KTZ = 15  # the 5 a_hist rows (a rank-5 term) are precomputed on host as ya
DINP = KT1 * 128
HT = HID // 128  # 4 hidden feature tiles
LT = L // 128  # 3 output feature tiles
ZROW0 = (HIST - 1) * L  # 1536: first row of z_last within x^T

# vecs columns: [b1 (4) | s0 (4) | c0 (4) | s1 (4) | c1 (4) | b3 (3)]
COL_B1 = 0
COL_S = lambda l: 4 + 8 * l
COL_C = lambda l: 8 + 8 * l
COL_B3 = 4 + 8 * NHL
NVEC = COL_B3 + LT


def build_bass(batch=B, dt_mode=DT_MODE):
    import concourse.bacc as bacc
    import concourse.tile as tile
    from concourse import mybir

    f32 = mybir.dt.float32
    mdt = mybir.dt.bfloat16 if dt_mode == "bf16" else mybir.dt.float32r
    Relu = mybir.ActivationFunctionType.Relu
    add = mybir.AluOpType.add

    widths = [NCH] * (batch // NCH)
    assert sum(widths) == batch

    nc = bacc.Bacc("TRN2", target_bir_lowering=False)
    xT = nc.declare_dram_parameter("xT", [DINP, batch], mdt, isOutput=False)
    zT = nc.declare_dram_parameter("zT", [L, batch], f32, isOutput=False)
    ya = nc.declare_dram_parameter("ya", [128, HT, batch], mdt, isOutput=False)
    w1 = nc.declare_dram_parameter("w1", [128, KT1, HID], mdt, isOutput=False)
    wh = nc.declare_dram_parameter("wh", [128, NHL, HT, HT, 128], mdt, isOutput=False)
    w3 = nc.declare_dram_parameter("w3", [128, HT, L], mdt, isOutput=False)
    vecs = nc.declare_dram_parameter("vecs", [128, NVEC], f32, isOutput=False)
    outT = nc.declare_dram_parameter("outT", [L, batch], f32, isOutput=True)

    with tile.TileContext(nc) as tc:
        with (
            tc.tile_pool(name="wt", bufs=1) as wpool,
            tc.tile_pool(name="x", bufs=2) as xpool,
            tc.tile_pool(name="z", bufs=2) as zpool,
            tc.tile_pool(name="h", bufs=3) as hpool,
            tc.tile_pool(name="o", bufs=3) as opool,
            tc.tile_pool(name="ps1", bufs=5, space="PSUM") as ps1pool,
            tc.tile_pool(name="psh", bufs=3, space="PSUM") as pshpool,
        ):
            # per-k-tile weight tiles so the first matmul only waits on its
            # own 256KB slice, not the whole 4MB preload
            w1_sb = []
            for kt in range(KTZ):
                t = wpool.tile([128, HID], mdt, tag=f"w1_{kt}")
                nc.sync.dma_start(out=t[:], in_=w1[:, kt, :])
                w1_sb.append(t)
            # allocate now, DMA after chunk-0's x tiles are queued so the
            # first layer-1 matmuls aren't stuck behind these preloads
            wh_sb = wpool.tile([128, NHL, HT, HT, 128], mdt, tag="wh")
            w3_sb = wpool.tile([128, HT, L], mdt, tag="w3")
            v_sb = wpool.tile([128, NVEC], f32, tag="vecs")

            b0 = 0
            for c, w_c in enumerate(widths):
                # chunk 0: x first (it gates the first matmul group; ya/z are
                # not read until the group completes). Steady state: small
                # ya/z streams first — the L1 psum recycle waits on the ya
                # add, and the out stage on z — don't queue them last.
                def emit_yaz():
                    ya_t = zpool.tile([128, HT, w_c], mdt, tag="ya")
                    nc.sync.dma_start(out=ya_t[:], in_=ya[:, :, b0 : b0 + w_c])
                    zts = []
                    for lt in range(LT):
                        zt = zpool.tile([128, w_c], f32, tag=f"z{lt}")
                        nc.sync.dma_start(
                            out=zt[:],
                            in_=zT[lt * 128 : (lt + 1) * 128, b0 : b0 + w_c],
                        )
                        zts.append(zt)
                    return [ya_t[:, ht, :] for ht in range(HT)], zts

                def emit_x():
                    xts = []
                    for kt in range(KTZ):
                        xt = xpool.tile([128, w_c], mdt, tag=f"x{kt}")
                        nc.sync.dma_start(
                            out=xt[:],
                            in_=xT[kt * 128 : (kt + 1) * 128, b0 : b0 + w_c],
                        )
                        xts.append(xt)
                    return xts

                if c == 0:
                    xts = emit_x()
                    yas, zts = emit_yaz()
                else:
                    yas, zts = emit_yaz()
                    xts = emit_x()

                if c == 0:
                    nc.sync.dma_start(out=wh_sb[:], in_=wh[:])
                    nc.sync.dma_start(out=w3_sb[:], in_=w3[:])
                    nc.sync.dma_start(out=v_sb[:], in_=vecs[:])

                # ---- layer 1: h1 = relu(W1^T x + b1), [512, NCH] ----
                h1 = []
                for ht in range(HT):
                    ps = ps1pool.tile([128, w_c], f32, tag="ps1")
                    for kt in range(KTZ):
                        nc.tensor.matmul(
                            ps[:],
                            w1_sb[kt][:, ht * 128 : (ht + 1) * 128],
                            xts[kt][:],
                            start=(kt == 0),
                            stop=(kt == KTZ - 1),
                        )
                    tsb = hpool.tile([128, w_c], f32, tag=f"t1_{ht}")
                    nc.vector.tensor_tensor(tsb[:], ps[:], yas[ht], add)
                    hsb = hpool.tile([128, w_c], mdt, tag=f"h1_{ht}")
                    nc.scalar.activation(
                        hsb[:], tsb[:], Relu, bias=v_sb[:, COL_B1 + ht : COL_B1 + ht + 1]
                    )
                    h1.append(hsb)

                # ---- hidden layers: h = relu((h @ Wh[l]) * s_l + c_l) ----
                hin = h1
                for l in range(NHL):
                    hout = []
                    for mt in range(HT):
                        ps = pshpool.tile([128, w_c], f32, tag="ps2")
                        for kt in range(HT):
                            nc.tensor.matmul(
                                ps[:],
                                wh_sb[:, l, kt, mt, :],
                                hin[kt][:],
                                start=(kt == 0),
                                stop=(kt == HT - 1),
                            )
                        hsb = hpool.tile([128, w_c], mdt, tag=f"h{l + 2}_{mt}")
                        nc.scalar.activation(
                            hsb[:],
                            ps[:],
                            Relu,
                            bias=v_sb[:, COL_C(l) + mt : COL_C(l) + mt + 1],
                            scale=v_sb[:, COL_S(l) + mt : COL_S(l) + mt + 1],
                        )
                        hout.append(hsb)
                    hin = hout

                # ---- out: delta^T = W3^T h + b3; out = delta^T + zlast^T ----
                for lt in range(LT):
                    ps = ps1pool.tile([128, w_c], f32, tag="ps1")
                    for kt in range(HT):
                        nc.tensor.matmul(
                            ps[:],
                            w3_sb[:, kt, lt * 128 : (lt + 1) * 128],
                            hin[kt][:],
                            start=(kt == 0),
                            stop=(kt == HT - 1),
                        )
                    ot = opool.tile([128, w_c], f32, tag=f"o{lt}")
                    nc.vector.tensor_tensor(ot[:], ps[:], zts[lt][:], add)
                    nc.sync.dma_start(
                        out=outT[lt * 128 : (lt + 1) * 128, b0 : b0 + w_c], in_=ot[:]
                    )
                b0 += w_c
    nc.compile()
    return nc


def _mdt_np(dt_mode):
    if dt_mode == "bf16":
        import ml_dtypes

        return ml_dtypes.bfloat16
    return np.float32


def prep_core_inputs(
    z_hist, a_hist, W1, b1, Wh, bh, gamma, beta, rmean, rvar, W3, b3, dt_mode=DT_MODE
):
    """Host-side shard prep: returns per-model input dicts (xT/zT shared)."""
    mnp = _mdt_np(dt_mode)
    batch = z_hist.shape[0]
    x = np.concatenate(
        [z_hist.reshape(batch, -1), a_hist.reshape(batch, -1)], axis=1
    ).astype(np.float32)
    xT = np.zeros((DINP, batch), mnp)
    xT[:DIN] = x.T.astype(mnp)
    a_flat = x[:, KTZ * 128 :]  # [batch, 5]
    z_lastT = np.ascontiguousarray(x.T[ZROW0 : ZROW0 + L]).astype(np.float32)

    rstd = 1.0 / np.sqrt(rvar.astype(np.float64) + EPS)  # [NHL, M, HID]
    s_aff = (gamma * rstd).astype(np.float32)
    c_aff = ((bh - rmean) * gamma * rstd + beta).astype(np.float32)

    in_maps = []
    for m in range(M):
        w1p = np.zeros((DINP, HID), np.float32)
        w1p[:DIN] = W1[m]
        w1h = np.ascontiguousarray(
            w1p.reshape(KT1, 128, HID).transpose(1, 0, 2)
        ).astype(mnp)  # [128, KT1, HID]

        whh = np.ascontiguousarray(
            Wh[:, m].reshape(NHL, HT, 128, HT, 128).transpose(2, 0, 1, 3, 4)
        ).astype(mnp)  # [128, NHL, kt, mt, 128]

        w3h = np.ascontiguousarray(W3[m].reshape(HT, 128, L).transpose(1, 0, 2)).astype(
            mnp
        )  # [128, HT, L]

        vecs = np.zeros((128, NVEC), np.float32)
        vecs[:, COL_B1 : COL_B1 + HT] = b1[m].reshape(HT, 128).T
        for l in range(NHL):
            vecs[:, COL_S(l) : COL_S(l) + HT] = s_aff[l, m].reshape(HT, 128).T
            vecs[:, COL_C(l) : COL_C(l) + HT] = c_aff[l, m].reshape(HT, 128).T
        vecs[:, COL_B3 : COL_B3 + LT] = b3[m].reshape(LT, 128).T

        zT = z_lastT + b3[m][:, None]  # fold b3 into the residual stream
        y_a = (a_flat @ W1[m][KTZ * 128 :]).T  # [HID, batch] fp32
        yah = np.ascontiguousarray(
            y_a.reshape(HT, 128, batch).transpose(1, 0, 2)
        ).astype(mnp)  # [128, HT, batch]
        in_maps.append(
            {"xT": xT, "zT": zT, "ya": yah, "w1": w1h, "wh": whh, "w3": w3h,
             "vecs": vecs}
        )
    return in_maps


def _reset_device():
    """Clear any exec-unit wedge a previous (profiled) session left behind."""
    try:
        import ctypes

        import jax

        jax.devices()
        lib = ctypes.CDLL("/opt/axon/libaxon_pjrt.so")
        if hasattr(lib, "axon_reset"):
            lib.axon_reset.restype = ctypes.c_int64
            lib.axon_reset()
    except Exception:
        pass


def kernel(**inputs):
    inputs = {k: np.asarray(v) for k, v in inputs.items()}
    in_maps = prep_core_inputs(**inputs)
    nc = build_bass(B)

    from concourse import bass_utils

    _reset_device()
    res = bass_utils.run_bass_kernel_spmd(nc, in_maps, core_ids=list(range(M)))
    out = np.stack(
        [np.ascontiguousarray(res.results[m]["outT"].T) for m in range(M)]
    )  # [M, B, L]
    return out.astype(np.float32)



# revision 4
# speedup vs baseline: 1.5743x; 1.5743x over previous
"""Trainium2 Bass kernel for nn_EnsembleTransitionModel.

Sharding: model-parallel (expert-parallel). M=8 ensemble members across 8
NeuronCores; each core runs one full MLP over the whole batch. Inputs are
replicated, per-model weights are sharded.

All four matmul layers run in fp8 (e4m3) DoubleRow perf mode: each matmul
instruction contracts K=256 (two 128-k-tiles packed as pairs in the free
dim of both operands) at 2 MACs/cell/cycle — 2x bf16 throughput. fp32
accumulation in PSUM.

Precision plan (measured rel-err ~2.2e-3 vs the 2e-2 gate):
  - W1, Wh are scaled x64 before fp8 quantization (raw weights ~N(0,0.02)
    sit in e4m3's subnormal range); the 1/64 rides the existing per-feature
    affine applied by the scalar-engine Relu activation out of PSUM.
  - W3 stays unscaled (error is the same either way) so the output stage is
    a single fp32 vector add of the residual stream, exactly as in bf16.
  - The residual z_last (+ b3 folded) rides a separate fp32 path so the
    dominant output term stays full precision.
  - DIN=1925 is zero-padded to 2048 so the 5 a_hist rows fold into the main
    L1 matmul (no separate host-computed rank-5 term).

Layouts: activations feature-major (x^T: [features, batch]); x is packed
chunk-major in DRAM ([128, chunk, kpair, 2, 512]) so each chunk's DMA is
one fully-contiguous 8KB-per-partition transfer.
"""

import os
import sys

import numpy as np

for _p in ("/opt/trn_rl_repo", "/root/.axon_site/_ro/trn_rl_repo"):
    if os.path.isdir(_p) and _p not in sys.path:
        sys.path.insert(0, _p)

M = 8
B = 16384
HIST = 5
L = 384
A = 1
HID = 512
NHL = 2
DIN = L * HIST + A * HIST  # 1925
EPS = 1e-5

WS = 64.0  # fp8 weight pre-scale for W1/Wh (compensated in the affine)

NCH = 512  # batch columns per chunk (= max fp32 moving dim = 1 PSUM bank)
DINP = 2048  # DIN zero-padded to 16 k-tiles
KP1 = DINP // 256  # 8 L1 k-pairs (DoubleRow consumes 2 k-tiles per matmul)
HT = HID // 128  # 4 hidden feature tiles
HP = HT // 2  # 2 hidden k-pairs
LT = L // 128  # 3 output feature tiles
ZROW0 = (HIST - 1) * L  # 1536: first row of z_last within x^T

# vecs columns: [b1 (4) | s0 (4) | c0 (4) | s1 (4) | c1 (4) | sL1 (1)]
COL_B1 = 0
COL_S = lambda l: 4 + 8 * l
COL_C = lambda l: 8 + 8 * l
COL_SL1 = 4 + 8 * NHL
NVEC = COL_SL1 + 1


def build_bass(batch=B):
    import concourse.bacc as bacc
    import concourse.tile as tile
    from concourse import mybir

    f32 = mybir.dt.float32
    f8 = mybir.dt.float8e4
    DR = mybir.MatmulPerfMode.DoubleRow
    Relu = mybir.ActivationFunctionType.Relu
    add = mybir.AluOpType.add

    nchunk = batch // NCH
    assert nchunk * NCH == batch

    nc = bacc.Bacc("TRN2", target_bir_lowering=False)
    xT = nc.declare_dram_parameter("xT", [128, nchunk, KP1, 2, NCH], f8, isOutput=False)
    zT = nc.declare_dram_parameter("zT", [128, nchunk, LT, NCH], f32, isOutput=False)
    w1 = nc.declare_dram_parameter("w1", [128, KP1, 2, HID], f8, isOutput=False)
    wh = nc.declare_dram_parameter("wh", [128, NHL, HP, 2, HT, 128], f8, isOutput=False)
    w3 = nc.declare_dram_parameter("w3", [128, HP, 2, L], f8, isOutput=False)
    vecs = nc.declare_dram_parameter("vecs", [128, NVEC], f32, isOutput=False)
    outT = nc.declare_dram_parameter("outT", [L, batch], f32, isOutput=True)

    with tile.TileContext(nc) as tc:
        with (
            tc.tile_pool(name="wt", bufs=1) as wpool,
            tc.tile_pool(name="x", bufs=2) as xpool,
            tc.tile_pool(name="z", bufs=2) as zpool,
            tc.tile_pool(name="h", bufs=2) as hpool,
            tc.tile_pool(name="o", bufs=3) as opool,
            tc.tile_pool(name="ps1", bufs=5, space="PSUM") as ps1pool,
            tc.tile_pool(name="psh", bufs=3, space="PSUM") as pshpool,
        ):
            # per-k-pair W1 tiles so the first matmul only waits on its own
            # 128KB slice, not the whole 1MB preload
            w1_sb = []
            for kp in range(KP1):
                t = wpool.tile([128, 2, HID], f8, tag=f"w1_{kp}")
                nc.sync.dma_start(out=t[:], in_=w1[:, kp])
                w1_sb.append(t)
            # allocate now, DMA after chunk-0's x tile is queued so the
            # first layer-1 matmuls aren't stuck behind these preloads
            wh_sb = wpool.tile([128, NHL, HP, 2, HT, 128], f8, tag="wh")
            w3_sb = wpool.tile([128, HP, 2, L], f8, tag="w3")
            v_sb = wpool.tile([128, NVEC], f32, tag="vecs")

            for c in range(nchunk):
                x_t = xpool.tile([128, KP1, 2, NCH], f8, tag="x")
                nc.sync.dma_start(out=x_t[:], in_=xT[:, c])
                z_t = zpool.tile([128, LT, NCH], f32, tag="z")
                nc.sync.dma_start(out=z_t[:], in_=zT[:, c])

                if c == 0:
                    nc.sync.dma_start(out=wh_sb[:], in_=wh[:])
                    nc.sync.dma_start(out=w3_sb[:], in_=w3[:])
                    nc.sync.dma_start(out=v_sb[:], in_=vecs[:])

                # ---- layer 1: h1 = relu((W1^T x)/64 + b1), [512, NCH] ----
                h1 = [
                    hpool.tile([128, 2, NCH], f8, tag=f"h1_{p}", name=f"h1_{p}")
                    for p in range(HP)
                ]
                for ht in range(HT):
                    ps = ps1pool.tile([128, NCH], f32, tag="ps1")
                    for kp in range(KP1):
                        nc.tensor.matmul(
                            ps[:],
                            w1_sb[kp][:, :, ht * 128 : (ht + 1) * 128],
                            x_t[:, kp],
                            start=(kp == 0),
                            stop=(kp == KP1 - 1),
                            perf_mode=DR,
                        )
                    nc.scalar.activation(
                        h1[ht // 2][:, ht % 2, :],
                        ps[:],
                        Relu,
                        bias=v_sb[:, COL_B1 + ht : COL_B1 + ht + 1],
                        scale=v_sb[:, COL_SL1 : COL_SL1 + 1],
                    )

                # ---- hidden: h = relu((h @ Wh[l]) * s_l/64 + c_l) ----
                hin = h1
                for l in range(NHL):
                    hout = [
                        hpool.tile(
                            [128, 2, NCH], f8, tag=f"h{l + 2}_{p}", name=f"h{l + 2}_{p}"
                        )
                        for p in range(HP)
                    ]
                    for mt in range(HT):
                        ps = pshpool.tile([128, NCH], f32, tag="ps2")
                        for kp in range(HP):
                            nc.tensor.matmul(
                                ps[:],
                                wh_sb[:, l, kp, :, mt, :],
                                hin[kp][:],
                                start=(kp == 0),
                                stop=(kp == HP - 1),
                                perf_mode=DR,
                            )
                        nc.scalar.activation(
                            hout[mt // 2][:, mt % 2, :],
                            ps[:],
                            Relu,
                            bias=v_sb[:, COL_C(l) + mt : COL_C(l) + mt + 1],
                            scale=v_sb[:, COL_S(l) + mt : COL_S(l) + mt + 1],
                        )
                    hin = hout

                # ---- out: delta^T = W3^T h; out = delta^T + (zlast+b3)^T ----
                for lt in range(LT):
                    ps = ps1pool.tile([128, NCH], f32, tag="ps1")
                    for kp in range(HP):
                        nc.tensor.matmul(
                            ps[:],
                            w3_sb[:, kp, :, lt * 128 : (lt + 1) * 128],
                            hin[kp][:],
                            start=(kp == 0),
                            stop=(kp == HP - 1),
                            perf_mode=DR,
                        )
                    ot = opool.tile([128, NCH], f32, tag=f"o{lt}")
                    nc.vector.tensor_tensor(ot[:], ps[:], z_t[:, lt, :], add)
                    nc.sync.dma_start(
                        out=outT[lt * 128 : (lt + 1) * 128, c * NCH : (c + 1) * NCH],
                        in_=ot[:],
                    )
    nc.compile()
    return nc


def _f8():
    import ml_dtypes

    return ml_dtypes.float8_e4m3


def prep_core_inputs(
    z_hist, a_hist, W1, b1, Wh, bh, gamma, beta, rmean, rvar, W3, b3
):
    """Host-side shard prep: returns per-model input dicts (xT shared)."""
    f8 = _f8()
    batch = z_hist.shape[0]
    nchunk = batch // NCH
    x = np.concatenate(
        [z_hist.reshape(batch, -1), a_hist.reshape(batch, -1)], axis=1
    ).astype(np.float32)
    xpad = np.zeros((batch, DINP), np.float32)
    xpad[:, :DIN] = x
    xq = xpad.astype(f8)  # quantize once, then pure byte shuffles
    xT8 = np.ascontiguousarray(
        xq.reshape(nchunk, NCH, DINP // 128, 128).transpose(3, 0, 2, 1)
    ).reshape(128, nchunk, KP1, 2, NCH)

    z_last = z_hist[:, -1, :].astype(np.float32)  # [batch, L]

    rstd = 1.0 / np.sqrt(rvar.astype(np.float64) + EPS)  # [NHL, M, HID]
    s_aff = (gamma * rstd).astype(np.float32)
    c_aff = ((bh - rmean) * gamma * rstd + beta).astype(np.float32)

    in_maps = []
    for m in range(M):
        w1p = np.zeros((DINP, HID), np.float32)
        w1p[:DIN] = W1[m] * WS
        w1h = np.ascontiguousarray(
            w1p.astype(f8).reshape(DINP // 128, 128, HID).transpose(1, 0, 2)
        ).reshape(128, KP1, 2, HID)

        whh = np.ascontiguousarray(
            (Wh[:, m] * WS)
            .astype(np.float32)
            .astype(f8)
            .reshape(NHL, HT, 128, HT, 128)
            .transpose(2, 0, 1, 3, 4)
        ).reshape(128, NHL, HP, 2, HT, 128)

        w3h = np.ascontiguousarray(
            W3[m].astype(np.float32).astype(f8).reshape(HT, 128, L).transpose(1, 0, 2)
        ).reshape(128, HP, 2, L)

        vecs = np.zeros((128, NVEC), np.float32)
        vecs[:, COL_B1 : COL_B1 + HT] = b1[m].reshape(HT, 128).T
        for l in range(NHL):
            vecs[:, COL_S(l) : COL_S(l) + HT] = (s_aff[l, m] / WS).reshape(HT, 128).T
            vecs[:, COL_C(l) : COL_C(l) + HT] = c_aff[l, m].reshape(HT, 128).T
        vecs[:, COL_SL1] = 1.0 / WS

        zb = z_last + b3[m][None, :]  # fold b3 into the residual stream
        zTm = np.ascontiguousarray(
            zb.reshape(nchunk, NCH, LT, 128).transpose(3, 0, 2, 1)
        )  # [128, nchunk, LT, NCH]
        in_maps.append(
            {"xT": xT8, "zT": zTm, "w1": w1h, "wh": whh, "w3": w3h, "vecs": vecs}
        )
    return in_maps


def _reset_device():
    """Clear any exec-unit wedge a previous (profiled) session left behind."""
    try:
        import ctypes

        import jax

        jax.devices()
        lib = ctypes.CDLL("/opt/axon/libaxon_pjrt.so")
        if hasattr(lib, "axon_reset"):
            lib.axon_reset.restype = ctypes.c_int64
            lib.axon_reset()
    except Exception:
        pass


def kernel(**inputs):
    inputs = {k: np.asarray(v) for k, v in inputs.items()}
    in_maps = prep_core_inputs(**inputs)
    nc = build_bass(B)

    from concourse import bass_utils

    _reset_device()
    res = bass_utils.run_bass_kernel_spmd(nc, in_maps, core_ids=list(range(M)))
    out = np.stack(
        [np.ascontiguousarray(res.results[m]["outT"].T) for m in range(M)]
    )  # [M, B, L]
    return out.astype(np.float32)


# revision 9
# speedup vs baseline: 1.6669x; 1.0588x over previous
"""Trainium2 Bass kernel for nn_EnsembleTransitionModel.

Sharding: model-parallel (expert-parallel). M=8 ensemble members across 8
NeuronCores; each core runs one full MLP over the whole batch. Inputs are
replicated, per-model weights are sharded.

All four matmul layers run in fp8 (e4m3) DoubleRow perf mode: each matmul
instruction contracts K=256 (two 128-k-tiles packed as pairs in the free
dim of both operands) at 2 MACs/cell/cycle — 2x bf16 throughput. fp32
accumulation in PSUM.

Precision plan (measured rel-err ~2.2e-3 vs the 2e-2 gate):
  - W1, Wh are scaled x64 before fp8 quantization (raw weights ~N(0,0.02)
    sit in e4m3's subnormal range); the 1/64 rides the existing per-feature
    affine applied by the scalar-engine Relu activation out of PSUM.
  - W3 stays unscaled (error is the same either way) so the output stage is
    a single fp32 vector add of the residual stream, exactly as in bf16.
  - The residual z_last (+ b3 folded) rides a separate fp32 path so the
    dominant output term stays full precision.
  - DIN=1925 is zero-padded to 2048 so the 5 a_hist rows fold into the main
    L1 matmul (no separate host-computed rank-5 term).

Layouts: activations feature-major (x^T: [features, batch]); x is packed
chunk-major in DRAM ([128, chunk, kpair, 2, 512]) so each chunk's DMA is
one fully-contiguous 8KB-per-partition transfer.
"""

import os
import sys

import numpy as np

for _p in ("/opt/trn_rl_repo", "/root/.axon_site/_ro/trn_rl_repo"):
    if os.path.isdir(_p) and _p not in sys.path:
        sys.path.insert(0, _p)

M = 8
B = 16384
HIST = 5
L = 384
A = 1
HID = 512
NHL = 2
DIN = L * HIST + A * HIST  # 1925
EPS = 1e-5

WS = 64.0  # fp8 weight pre-scale for W1/Wh (compensated in the affine)

NCH = 512  # batch columns per chunk (= max fp32 moving dim = 1 PSUM bank)
DINP = 2048  # DIN zero-padded to 16 k-tiles
KP1 = DINP // 256  # 8 L1 k-pairs (DoubleRow consumes 2 k-tiles per matmul)
HT = HID // 128  # 4 hidden feature tiles
HP = HT // 2  # 2 hidden k-pairs
LT = L // 128  # 3 output feature tiles
ZROW0 = (HIST - 1) * L  # 1536: first row of z_last within x^T

# vecs columns: [b1 (4) | s0 (4) | c0 (4) | s1 (4) | c1 (4) | sL1 (1)]
COL_B1 = 0
COL_S = lambda l: 4 + 8 * l
COL_C = lambda l: 8 + 8 * l
COL_SL1 = 4 + 8 * NHL
NVEC = COL_SL1 + 1


def build_bass(batch=B, zero_bias=True):
    """zero_bias=True (true for this model instance: b1/bh/beta/rmean all
    zero) routes half the activations to the Vector engine as a one-pass
    relu(scale*psum) tensor_scalar, halving the end-of-layer activation
    barrier the PE waits on. With nonzero biases everything stays on the
    Scalar engine's general affine activation path."""
    import concourse.bacc as bacc
    import concourse.tile as tile
    from concourse import mybir

    f32 = mybir.dt.float32
    f8 = mybir.dt.float8e4
    DR = mybir.MatmulPerfMode.DoubleRow
    Relu = mybir.ActivationFunctionType.Relu
    add = mybir.AluOpType.add
    mult = mybir.AluOpType.mult
    maxop = mybir.AluOpType.max

    nchunk = batch // NCH
    assert nchunk * NCH == batch

    nc = bacc.Bacc("TRN2", target_bir_lowering=False)
    xT = nc.declare_dram_parameter("xT", [128, nchunk, KP1, 2, NCH], f8, isOutput=False)
    zT = nc.declare_dram_parameter("zT", [128, nchunk, LT, NCH], f32, isOutput=False)
    w1 = nc.declare_dram_parameter("w1", [128, KP1, 2, HID], f8, isOutput=False)
    wh = nc.declare_dram_parameter("wh", [128, NHL, HP, 2, HT, 128], f8, isOutput=False)
    w3 = nc.declare_dram_parameter("w3", [128, HP, 2, L], f8, isOutput=False)
    vecs = nc.declare_dram_parameter("vecs", [128, NVEC], f32, isOutput=False)
    outT = nc.declare_dram_parameter("outT", [L, batch], f32, isOutput=True)

    with tile.TileContext(nc) as tc:
        with (
            tc.tile_pool(name="wt", bufs=1) as wpool,
            tc.tile_pool(name="x", bufs=2) as xpool,
            tc.tile_pool(name="z", bufs=2) as zpool,
            tc.tile_pool(name="h", bufs=2) as hpool,
            tc.tile_pool(name="o", bufs=3) as opool,
            tc.tile_pool(name="ps1", bufs=5, space="PSUM") as ps1pool,
            tc.tile_pool(name="psh", bufs=3, space="PSUM") as pshpool,
        ):
            # per-k-pair W1 tiles so the first matmul only waits on its own
            # 128KB slice, not the whole 1MB preload
            w1_sb = []
            for kp in range(KP1):
                t = wpool.tile([128, 2, HID], f8, tag=f"w1_{kp}")
                nc.sync.dma_start(out=t[:], in_=w1[:, kp])
                w1_sb.append(t)
            # allocate now, DMA after chunk-0's x tile is queued so the
            # first layer-1 matmuls aren't stuck behind these preloads
            wh_sb = wpool.tile([128, NHL, HP, 2, HT, 128], f8, tag="wh")
            w3_sb = wpool.tile([128, HP, 2, L], f8, tag="w3")
            v_sb = wpool.tile([128, NVEC], f32, tag="vecs")

            for c in range(nchunk):
                x_t = xpool.tile([128, KP1, 2, NCH], f8, tag="x")
                if c == 0:
                    # split so the first matmul gates on one 128KB slice,
                    # not the whole 1MB chunk transfer
                    for kp in range(KP1):
                        nc.sync.dma_start(out=x_t[:, kp], in_=xT[:, c, kp])
                else:
                    nc.sync.dma_start(out=x_t[:], in_=xT[:, c])
                z_t = zpool.tile([128, LT, NCH], f32, tag="z")
                nc.sync.dma_start(out=z_t[:], in_=zT[:, c])

                if c == 0:
                    nc.sync.dma_start(out=wh_sb[:], in_=wh[:])
                    nc.sync.dma_start(out=w3_sb[:], in_=w3[:])
                    nc.sync.dma_start(out=v_sb[:], in_=vecs[:])

                # ---- layer 1: h1 = relu((W1^T x)/64 + b1), [512, NCH] ----
                h1 = [
                    hpool.tile([128, 2, NCH], f8, tag=f"h1_{p}", name=f"h1_{p}")
                    for p in range(HP)
                ]
                for ht in range(HT):
                    ps = ps1pool.tile([128, NCH], f32, tag="ps1")
                    for kp in range(KP1):
                        nc.tensor.matmul(
                            ps[:],
                            w1_sb[kp][:, :, ht * 128 : (ht + 1) * 128],
                            x_t[:, kp],
                            start=(kp == 0),
                            stop=(kp == KP1 - 1),
                            perf_mode=DR,
                        )
                    if zero_bias and ht % 2 == 1:
                        nc.vector.tensor_scalar(
                            h1[ht // 2][:, ht % 2, :],
                            ps[:],
                            1.0 / WS,
                            0.0,
                            mult,
                            maxop,
                        )
                    else:
                        nc.scalar.activation(
                            h1[ht // 2][:, ht % 2, :],
                            ps[:],
                            Relu,
                            bias=v_sb[:, COL_B1 + ht : COL_B1 + ht + 1],
                            scale=v_sb[:, COL_SL1 : COL_SL1 + 1],
                        )

                # ---- hidden: h = relu((h @ Wh[l]) * s_l/64 + c_l) ----
                hin = h1
                for l in range(NHL):
                    hout = [
                        hpool.tile(
                            [128, 2, NCH], f8, tag=f"h{l + 2}_{p}", name=f"h{l + 2}_{p}"
                        )
                        for p in range(HP)
                    ]
                    for mt in range(HT):
                        ps = pshpool.tile([128, NCH], f32, tag="ps2")
                        for kp in range(HP):
                            nc.tensor.matmul(
                                ps[:],
                                wh_sb[:, l, kp, :, mt, :],
                                hin[kp][:],
                                start=(kp == 0),
                                stop=(kp == HP - 1),
                                perf_mode=DR,
                            )
                        if zero_bias and mt % 2 == 1:
                            nc.vector.tensor_scalar(
                                hout[mt // 2][:, mt % 2, :],
                                ps[:],
                                v_sb[:, COL_S(l) + mt : COL_S(l) + mt + 1],
                                0.0,
                                mult,
                                maxop,
                            )
                        else:
                            nc.scalar.activation(
                                hout[mt // 2][:, mt % 2, :],
                                ps[:],
                                Relu,
                                bias=v_sb[:, COL_C(l) + mt : COL_C(l) + mt + 1],
                                scale=v_sb[:, COL_S(l) + mt : COL_S(l) + mt + 1],
                            )
                    hin = hout

                # ---- out: delta^T = W3^T h; out = delta^T + (zlast+b3)^T ----
                for lt in range(LT):
                    ps = ps1pool.tile([128, NCH], f32, tag="ps1")
                    for kp in range(HP):
                        nc.tensor.matmul(
                            ps[:],
                            w3_sb[:, kp, :, lt * 128 : (lt + 1) * 128],
                            hin[kp][:],
                            start=(kp == 0),
                            stop=(kp == HP - 1),
                            perf_mode=DR,
                        )
                    ot = opool.tile([128, NCH], f32, tag=f"o{lt}")
                    nc.vector.tensor_tensor(ot[:], ps[:], z_t[:, lt, :], add)
                    nc.sync.dma_start(
                        out=outT[lt * 128 : (lt + 1) * 128, c * NCH : (c + 1) * NCH],
                        in_=ot[:],
                    )
    nc.compile()
    return nc


def _f8():
    import ml_dtypes

    return ml_dtypes.float8_e4m3


def prep_core_inputs(
    z_hist, a_hist, W1, b1, Wh, bh, gamma, beta, rmean, rvar, W3, b3
):
    """Host-side shard prep: returns per-model input dicts (xT shared)."""
    f8 = _f8()
    batch = z_hist.shape[0]
    nchunk = batch // NCH
    x = np.concatenate(
        [z_hist.reshape(batch, -1), a_hist.reshape(batch, -1)], axis=1
    ).astype(np.float32)
    xpad = np.zeros((batch, DINP), np.float32)
    xpad[:, :DIN] = x
    xq = xpad.astype(f8)  # quantize once, then pure byte shuffles
    xT8 = np.ascontiguousarray(
        xq.reshape(nchunk, NCH, DINP // 128, 128).transpose(3, 0, 2, 1)
    ).reshape(128, nchunk, KP1, 2, NCH)

    z_last = z_hist[:, -1, :].astype(np.float32)  # [batch, L]

    rstd = 1.0 / np.sqrt(rvar.astype(np.float64) + EPS)  # [NHL, M, HID]
    s_aff = (gamma * rstd).astype(np.float32)
    c_aff = ((bh - rmean) * gamma * rstd + beta).astype(np.float32)

    in_maps = []
    for m in range(M):
        w1p = np.zeros((DINP, HID), np.float32)
        w1p[:DIN] = W1[m] * WS
        w1h = np.ascontiguousarray(
            w1p.astype(f8).reshape(DINP // 128, 128, HID).transpose(1, 0, 2)
        ).reshape(128, KP1, 2, HID)

        whh = np.ascontiguousarray(
            (Wh[:, m] * WS)
            .astype(np.float32)
            .astype(f8)
            .reshape(NHL, HT, 128, HT, 128)
            .transpose(2, 0, 1, 3, 4)
        ).reshape(128, NHL, HP, 2, HT, 128)

        w3h = np.ascontiguousarray(
            W3[m].astype(np.float32).astype(f8).reshape(HT, 128, L).transpose(1, 0, 2)
        ).reshape(128, HP, 2, L)

        vecs = np.zeros((128, NVEC), np.float32)
        vecs[:, COL_B1 : COL_B1 + HT] = b1[m].reshape(HT, 128).T
        for l in range(NHL):
            vecs[:, COL_S(l) : COL_S(l) + HT] = (s_aff[l, m] / WS).reshape(HT, 128).T
            vecs[:, COL_C(l) : COL_C(l) + HT] = c_aff[l, m].reshape(HT, 128).T
        vecs[:, COL_SL1] = 1.0 / WS

        zb = z_last + b3[m][None, :]  # fold b3 into the residual stream
        zTm = np.ascontiguousarray(
            zb.reshape(nchunk, NCH, LT, 128).transpose(3, 0, 2, 1)
        )  # [128, nchunk, LT, NCH]
        in_maps.append(
            {"xT": xT8, "zT": zTm, "w1": w1h, "wh": whh, "w3": w3h, "vecs": vecs}
        )
    return in_maps


def _reset_device():
    """Clear any exec-unit wedge a previous (profiled) session left behind."""
    try:
        import ctypes

        import jax

        jax.devices()
        lib = ctypes.CDLL("/opt/axon/libaxon_pjrt.so")
        if hasattr(lib, "axon_reset"):
            lib.axon_reset.restype = ctypes.c_int64
            lib.axon_reset()
    except Exception:
        pass


def is_zero_bias(inputs):
    """True iff every additive term of the per-layer affines is zero, i.e.
    the activations reduce to relu(scale * psum)."""
    return not (
        inputs["b1"].any()
        or inputs["bh"].any()
        or inputs["beta"].any()
        or inputs["rmean"].any()
    )


def kernel(**inputs):
    inputs = {k: np.asarray(v) for k, v in inputs.items()}
    in_maps = prep_core_inputs(**inputs)
    nc = build_bass(B, zero_bias=is_zero_bias(inputs))

    from concourse import bass_utils

    _reset_device()
    res = bass_utils.run_bass_kernel_spmd(nc, in_maps, core_ids=list(range(M)))
    out = np.stack(
        [np.ascontiguousarray(res.results[m]["outT"].T) for m in range(M)]
    )  # [M, B, L]
    return out.astype(np.float32)


# revision 15
# speedup vs baseline: 1.6772x; 1.0062x over previous
"""Trainium2 Bass kernel for nn_EnsembleTransitionModel.

Sharding: model-parallel (expert-parallel). M=8 ensemble members across 8
NeuronCores; each core runs one full MLP over the whole batch. Inputs are
replicated, per-model weights are sharded.

All four matmul layers run in fp8 (e4m3) DoubleRow perf mode: each matmul
instruction contracts K=256 (two 128-k-tiles packed as pairs in the free
dim of both operands) at 2 MACs/cell/cycle — 2x bf16 throughput. fp32
accumulation in PSUM.

Precision plan (measured rel-err ~2.2e-3 vs the 2e-2 gate):
  - W1, Wh are scaled x64 before fp8 quantization (raw weights ~N(0,0.02)
    sit in e4m3's subnormal range); the 1/64 rides the existing per-feature
    affine applied by the scalar-engine Relu activation out of PSUM.
  - W3 stays unscaled (error is the same either way) so the output stage is
    a single fp32 vector add of the residual stream, exactly as in bf16.
  - The residual z_last (+ b3 folded) rides a separate fp32 path so the
    dominant output term stays full precision.
  - DIN=1925 is zero-padded to 2048 so the 5 a_hist rows fold into the main
    L1 matmul (no separate host-computed rank-5 term).

Layouts: activations feature-major (x^T: [features, batch]); x is packed
chunk-major in DRAM ([128, chunk, kpair, 2, 512]) so each chunk's DMA is
one fully-contiguous 8KB-per-partition transfer.
"""

import os
import sys

import numpy as np

for _p in ("/opt/trn_rl_repo", "/root/.axon_site/_ro/trn_rl_repo"):
    if os.path.isdir(_p) and _p not in sys.path:
        sys.path.insert(0, _p)

M = 8
B = 16384
HIST = 5
L = 384
A = 1
HID = 512
NHL = 2
DIN = L * HIST + A * HIST  # 1925
EPS = 1e-5

WS = 64.0  # fp8 weight pre-scale for W1/Wh (compensated in the affine)

NCH = 512  # batch columns per chunk (= max fp32 moving dim = 1 PSUM bank)
DINP = 2048  # DIN zero-padded to 16 k-tiles
KP1 = DINP // 256  # 8 L1 k-pairs (DoubleRow consumes 2 k-tiles per matmul)
HT = HID // 128  # 4 hidden feature tiles
HP = HT // 2  # 2 hidden k-pairs
LT = L // 128  # 3 output feature tiles
ZROW0 = (HIST - 1) * L  # 1536: first row of z_last within x^T

# vecs columns: [b1 (4) | s0 (4) | c0 (4) | s1 (4) | c1 (4) | sL1 (1)]
COL_B1 = 0
COL_S = lambda l: 4 + 8 * l
COL_C = lambda l: 8 + 8 * l
COL_SL1 = 4 + 8 * NHL
NVEC = COL_SL1 + 1


def build_bass(batch=B, zero_bias=True):
    """zero_bias=True (true for this model instance: b1/bh/beta/rmean all
    zero) routes half the activations to the Vector engine as a one-pass
    relu(scale*psum) tensor_scalar, halving the end-of-layer activation
    barrier the PE waits on. With nonzero biases everything stays on the
    Scalar engine's general affine activation path."""
    import concourse.bacc as bacc
    import concourse.tile as tile
    from concourse import mybir

    f32 = mybir.dt.float32
    f8 = mybir.dt.float8e4
    DR = mybir.MatmulPerfMode.DoubleRow
    Relu = mybir.ActivationFunctionType.Relu
    add = mybir.AluOpType.add
    mult = mybir.AluOpType.mult
    maxop = mybir.AluOpType.max

    nchunk = batch // NCH
    assert nchunk * NCH == batch

    nc = bacc.Bacc("TRN2", target_bir_lowering=False)
    xT = nc.declare_dram_parameter("xT", [128, nchunk, KP1, 2, NCH], f8, isOutput=False)
    zT = nc.declare_dram_parameter("zT", [128, nchunk, LT, NCH], f32, isOutput=False)
    w1 = nc.declare_dram_parameter("w1", [128, KP1, 2, HID], f8, isOutput=False)
    wh = nc.declare_dram_parameter("wh", [128, NHL, HP, 2, HT, 128], f8, isOutput=False)
    w3 = nc.declare_dram_parameter("w3", [128, HP, 2, L], f8, isOutput=False)
    vecs = nc.declare_dram_parameter("vecs", [128, NVEC], f32, isOutput=False)
    outT = nc.declare_dram_parameter("outT", [L, batch], f32, isOutput=True)

    with tile.TileContext(nc) as tc:
        with (
            tc.tile_pool(name="wt", bufs=1) as wpool,
            tc.tile_pool(name="x", bufs=2) as xpool,
            tc.tile_pool(name="z", bufs=2) as zpool,
            tc.tile_pool(name="h", bufs=2) as hpool,
            tc.tile_pool(name="o", bufs=3) as opool,
            tc.tile_pool(name="ps1", bufs=4, space="PSUM") as ps1pool,
            tc.tile_pool(name="psh", bufs=1, space="PSUM") as pshpool,
        ):
            # per-k-pair W1 tiles so the first matmul only waits on its own
            # 128KB slice, not the whole 1MB preload (DMAs are interleaved
            # with chunk-0 x slices below, kp by kp)
            w1_sb = [
                wpool.tile([128, 2, HID], f8, tag=f"w1_{kp}", name=f"w1_{kp}")
                for kp in range(KP1)
            ]
            # allocate now, DMA after chunk-0's x tile is queued so the
            # first layer-1 matmuls aren't stuck behind these preloads
            wh_sb = wpool.tile([128, NHL, HP, 2, HT, 128], f8, tag="wh")
            w3_sb = wpool.tile([128, HP, 2, L], f8, tag="w3")
            v_sb = wpool.tile([128, NVEC], f32, tag="vecs")

            for c in range(nchunk):
                x_t = xpool.tile([128, KP1, 2, NCH], f8, tag="x")
                if c == 0:
                    # interleave w1/x slices so the first matmul gates on
                    # 256KB of DMA, not the whole 2MB preload
                    for kp in range(KP1):
                        nc.sync.dma_start(out=w1_sb[kp][:], in_=w1[:, kp])
                        nc.sync.dma_start(out=x_t[:, kp], in_=xT[:, c, kp])
                else:
                    nc.sync.dma_start(out=x_t[:], in_=xT[:, c])
                z_t = zpool.tile([128, LT, NCH], f32, tag="z")
                nc.sync.dma_start(out=z_t[:], in_=zT[:, c])

                if c == 0:
                    nc.sync.dma_start(out=wh_sb[:], in_=wh[:])
                    nc.sync.dma_start(out=w3_sb[:], in_=w3[:])
                    nc.sync.dma_start(out=v_sb[:], in_=vecs[:])

                # ---- layer 1: h1 = relu((W1^T x)/64 + b1), [512, NCH] ----
                h1 = [
                    hpool.tile([128, 2, NCH], f8, tag=f"h1_{p}", name=f"h1_{p}")
                    for p in range(HP)
                ]
                for ht in range(HT):
                    ps = ps1pool.tile([128, NCH], f32, tag="ps1")
                    for kp in range(KP1):
                        nc.tensor.matmul(
                            ps[:],
                            w1_sb[kp][:, :, ht * 128 : (ht + 1) * 128],
                            x_t[:, kp],
                            start=(kp == 0),
                            stop=(kp == KP1 - 1),
                            perf_mode=DR,
                        )
                    if zero_bias and ht % 2 == 1:
                        nc.vector.tensor_scalar(
                            h1[ht // 2][:, ht % 2, :],
                            ps[:],
                            1.0 / WS,
                            0.0,
                            mult,
                            maxop,
                        )
                    else:
                        nc.scalar.activation(
                            h1[ht // 2][:, ht % 2, :],
                            ps[:],
                            Relu,
                            bias=v_sb[:, COL_B1 + ht : COL_B1 + ht + 1],
                            scale=v_sb[:, COL_SL1 : COL_SL1 + 1],
                        )

                # ---- hidden: h = relu((h @ Wh[l]) * s_l/64 + c_l) ----
                # kp-outer: all four kp=0 matmuls (which need only the
                # early-finishing h pair 0) run before any kp=1 matmul, so
                # pair-1 activations get ~900ns to land without stalling PE
                hin = h1
                for l in range(NHL):
                    hout = [
                        hpool.tile(
                            [128, 2, NCH], f8, tag=f"h{l + 2}_{p}", name=f"h{l + 2}_{p}"
                        )
                        for p in range(HP)
                    ]
                    pss = [
                        pshpool.tile([128, NCH], f32, tag=f"ps2_{mt}", name=f"ps2_{mt}")
                        for mt in range(HT)
                    ]
                    for mt in range(HT):
                        nc.tensor.matmul(
                            pss[mt][:],
                            wh_sb[:, l, 0, :, mt, :],
                            hin[0][:],
                            start=True,
                            stop=False,
                            perf_mode=DR,
                        )
                    # kp=1 in order [1,3,0,2]: the pair-0 acts (mt0/mt1) gate
                    # the next layer's first matmul group — stop mt1 first so
                    # its Vector act starts immediately, mt0's Scalar act runs
                    # concurrently; pair-1 acts have an extra group of slack
                    for mt in (1, 3, 0, 2):
                        nc.tensor.matmul(
                            pss[mt][:],
                            wh_sb[:, l, 1, :, mt, :],
                            hin[1][:],
                            start=False,
                            stop=True,
                            perf_mode=DR,
                        )
                        if zero_bias and mt % 2 == 1:
                            nc.vector.tensor_scalar(
                                hout[mt // 2][:, mt % 2, :],
                                pss[mt][:],
                                v_sb[:, COL_S(l) + mt : COL_S(l) + mt + 1],
                                0.0,
                                mult,
                                maxop,
                            )
                        else:
                            nc.scalar.activation(
                                hout[mt // 2][:, mt % 2, :],
                                pss[mt][:],
                                Relu,
                                bias=v_sb[:, COL_C(l) + mt : COL_C(l) + mt + 1],
                                scale=v_sb[:, COL_S(l) + mt : COL_S(l) + mt + 1],
                            )
                    hin = hout

                # ---- out: delta^T = W3^T h; out = delta^T + (zlast+b3)^T ----
                pso = [
                    ps1pool.tile([128, NCH], f32, tag="ps1", name=f"pso_{lt}")
                    for lt in range(LT)
                ]
                for lt in range(LT):
                    nc.tensor.matmul(
                        pso[lt][:],
                        w3_sb[:, 0, :, lt * 128 : (lt + 1) * 128],
                        hin[0][:],
                        start=True,
                        stop=False,
                        perf_mode=DR,
                    )
                for lt in range(LT):
                    nc.tensor.matmul(
                        pso[lt][:],
                        w3_sb[:, 1, :, lt * 128 : (lt + 1) * 128],
                        hin[1][:],
                        start=False,
                        stop=True,
                        perf_mode=DR,
                    )
                    ot = opool.tile([128, NCH], f32, tag=f"o{lt}", name=f"o{lt}")
                    nc.vector.tensor_tensor(ot[:], pso[lt][:], z_t[:, lt, :], add)
                    nc.sync.dma_start(
                        out=outT[lt * 128 : (lt + 1) * 128, c * NCH : (c + 1) * NCH],
                        in_=ot[:],
                    )
    nc.compile()
    return nc


def _f8():
    import ml_dtypes

    return ml_dtypes.float8_e4m3


def prep_core_inputs(
    z_hist, a_hist, W1, b1, Wh, bh, gamma, beta, rmean, rvar, W3, b3
):
    """Host-side shard prep: returns per-model input dicts (xT shared)."""
    f8 = _f8()
    batch = z_hist.shape[0]
    nchunk = batch // NCH
    x = np.concatenate(
        [z_hist.reshape(batch, -1), a_hist.reshape(batch, -1)], axis=1
    ).astype(np.float32)
    xpad = np.zeros((batch, DINP), np.float32)
    xpad[:, :DIN] = x
    xq = xpad.astype(f8)  # quantize once, then pure byte shuffles
    xT8 = np.ascontiguousarray(
        xq.reshape(nchunk, NCH, DINP // 128, 128).transpose(3, 0, 2, 1)
    ).reshape(128, nchunk, KP1, 2, NCH)

    z_last = z_hist[:, -1, :].astype(np.float32)  # [batch, L]

    rstd = 1.0 / np.sqrt(rvar.astype(np.float64) + EPS)  # [NHL, M, HID]
    s_aff = (gamma * rstd).astype(np.float32)
    c_aff = ((bh - rmean) * gamma * rstd + beta).astype(np.float32)

    in_maps = []
    for m in range(M):
        w1p = np.zeros((DINP, HID), np.float32)
        w1p[:DIN] = W1[m] * WS
        w1h = np.ascontiguousarray(
            w1p.astype(f8).reshape(DINP // 128, 128, HID).transpose(1, 0, 2)
        ).reshape(128, KP1, 2, HID)

        whh = np.ascontiguousarray(
            (Wh[:, m] * WS)
            .astype(np.float32)
            .astype(f8)
            .reshape(NHL, HT, 128, HT, 128)
            .transpose(2, 0, 1, 3, 4)
        ).reshape(128, NHL, HP, 2, HT, 128)

        w3h = np.ascontiguousarray(
            W3[m].astype(np.float32).astype(f8).reshape(HT, 128, L).transpose(1, 0, 2)
        ).reshape(128, HP, 2, L)

        vecs = np.zeros((128, NVEC), np.float32)
        vecs[:, COL_B1 : COL_B1 + HT] = b1[m].reshape(HT, 128).T
        for l in range(NHL):
            vecs[:, COL_S(l) : COL_S(l) + HT] = (s_aff[l, m] / WS).reshape(HT, 128).T
            vecs[:, COL_C(l) : COL_C(l) + HT] = c_aff[l, m].reshape(HT, 128).T
        vecs[:, COL_SL1] = 1.0 / WS

        zb = z_last + b3[m][None, :]  # fold b3 into the residual stream
        zTm = np.ascontiguousarray(
            zb.reshape(nchunk, NCH, LT, 128).transpose(3, 0, 2, 1)
        )  # [128, nchunk, LT, NCH]
        in_maps.append(
            {"xT": xT8, "zT": zTm, "w1": w1h, "wh": whh, "w3": w3h, "vecs": vecs}
        )
    return in_maps


def _reset_device():
    """Clear any exec-unit wedge a previous (profiled) session left behind."""
    try:
        import ctypes

        import jax

        jax.devices()
        lib = ctypes.CDLL("/opt/axon/libaxon_pjrt.so")
        if hasattr(lib, "axon_reset"):
            lib.axon_reset.restype = ctypes.c_int64
            lib.axon_reset()
    except Exception:
        pass


def is_zero_bias(inputs):
    """True iff every additive term of the per-layer affines is zero, i.e.
    the activations reduce to relu(scale * psum)."""
    return not (
        inputs["b1"].any()
        or inputs["bh"].any()
        or inputs["beta"].any()
        or inputs["rmean"].any()
    )


def kernel(**inputs):
    inputs = {k: np.asarray(v) for k, v in inputs.items()}
    in_maps = prep_core_inputs(**inputs)
    nc = build_bass(B, zero_bias=is_zero_bias(inputs))

    from concourse import bass_utils

    _reset_device()
    res = bass_utils.run_bass_kernel_spmd(nc, in_maps, core_ids=list(range(M)))
    out = np.stack(
        [np.ascontiguousarray(res.results[m]["outT"].T) for m in range(M)]
    )  # [M, B, L]
    return out.astype(np.float32)


# revision 16
# speedup vs baseline: 1.7892x; 1.0668x over previous
"""Trainium2 Bass kernel for nn_EnsembleTransitionModel.

Sharding: model-parallel (expert-parallel). M=8 ensemble members across 8
NeuronCores; each core runs one full MLP over the whole batch. Inputs are
replicated, per-model weights are sharded.

All four matmul layers run in fp8 (e4m3) DoubleRow perf mode: each matmul
instruction contracts K=256 (two 128-k-tiles packed as pairs in the free
dim of both operands) at 2 MACs/cell/cycle — 2x bf16 throughput. fp32
accumulation in PSUM.

Precision plan (measured rel-err ~2.2e-3 vs the 2e-2 gate):
  - W1, Wh are scaled x64 before fp8 quantization (raw weights ~N(0,0.02)
    sit in e4m3's subnormal range); the 1/64 rides the existing per-feature
    affine applied by the scalar-engine Relu activation out of PSUM.
  - W3 stays unscaled (error is the same either way) so the output stage is
    a single fp32 vector add of the residual stream, exactly as in bf16.
  - The residual z_last (+ b3 folded) rides a separate fp32 path so the
    dominant output term stays full precision.
  - DIN=1925 is zero-padded to 2048 so the 5 a_hist rows fold into the main
    L1 matmul (no separate host-computed rank-5 term).

Layouts: activations feature-major (x^T: [features, batch]); x is packed
chunk-major in DRAM ([128, chunk, kpair, 2, 512]) so each chunk's DMA is
one fully-contiguous 8KB-per-partition transfer.
"""

import os
import sys

import numpy as np

for _p in ("/opt/trn_rl_repo", "/root/.axon_site/_ro/trn_rl_repo"):
    if os.path.isdir(_p) and _p not in sys.path:
        sys.path.insert(0, _p)

M = 8
B = 16384
HIST = 5
L = 384
A = 1
HID = 512
NHL = 2
DIN = L * HIST + A * HIST  # 1925
EPS = 1e-5

WS = 64.0  # fp8 weight pre-scale for W1/Wh (compensated in the affine)

NCH = 512  # batch columns per chunk (= max fp32 moving dim = 1 PSUM bank)
DINP = 2048  # DIN zero-padded to 16 k-tiles
KP1 = DINP // 256  # 8 L1 k-pairs (DoubleRow consumes 2 k-tiles per matmul)
HT = HID // 128  # 4 hidden feature tiles
HP = HT // 2  # 2 hidden k-pairs
LT = L // 128  # 3 output feature tiles
ZROW0 = (HIST - 1) * L  # 1536: first row of z_last within x^T

# vecs columns: [b1 (4) | s0 (4) | c0 (4) | s1 (4) | c1 (4) | sL1 (1)]
COL_B1 = 0
COL_S = lambda l: 4 + 8 * l
COL_C = lambda l: 8 + 8 * l
COL_SL1 = 4 + 8 * NHL
NVEC = COL_SL1 + 1


def build_bass(batch=B, zero_bias=True):
    """zero_bias=True (true for this model instance: b1/bh/beta/rmean all
    zero) routes half the activations to the Vector engine as a one-pass
    relu(scale*psum) tensor_scalar, halving the end-of-layer activation
    barrier the PE waits on. With nonzero biases everything stays on the
    Scalar engine's general affine activation path."""
    import concourse.bacc as bacc
    import concourse.tile as tile
    from concourse import mybir

    f32 = mybir.dt.float32
    f8 = mybir.dt.float8e4
    DR = mybir.MatmulPerfMode.DoubleRow
    Relu = mybir.ActivationFunctionType.Relu
    add = mybir.AluOpType.add
    mult = mybir.AluOpType.mult
    maxop = mybir.AluOpType.max

    nchunk = batch // NCH
    assert nchunk * NCH == batch

    nc = bacc.Bacc("TRN2", target_bir_lowering=False)
    xT = nc.declare_dram_parameter("xT", [128, nchunk, KP1, 2, NCH], f8, isOutput=False)
    zT = nc.declare_dram_parameter("zT", [128, nchunk, LT, NCH], f32, isOutput=False)
    w1 = nc.declare_dram_parameter("w1", [128, KP1, 2, HID], f8, isOutput=False)
    wh = nc.declare_dram_parameter("wh", [128, NHL, HP, 2, HT, 128], f8, isOutput=False)
    w3 = nc.declare_dram_parameter("w3", [128, HP, 2, L], f8, isOutput=False)
    vecs = nc.declare_dram_parameter("vecs", [128, NVEC], f32, isOutput=False)
    outT = nc.declare_dram_parameter("outT", [L, batch], f32, isOutput=True)

    with tile.TileContext(nc) as tc:
        with (
            tc.tile_pool(name="wt", bufs=1) as wpool,
            tc.tile_pool(name="x", bufs=3) as xpool,
            tc.tile_pool(name="z", bufs=2) as zpool,
            tc.tile_pool(name="h", bufs=2) as hpool,
            tc.tile_pool(name="o", bufs=3) as opool,
            tc.tile_pool(name="ps1", bufs=4, space="PSUM") as ps1pool,
            tc.tile_pool(name="psh", bufs=1, space="PSUM") as pshpool,
        ):
            # per-k-pair W1 tiles so the first matmul only waits on its own
            # 128KB slice, not the whole 1MB preload (DMAs are interleaved
            # with chunk-0 x slices, kp by kp)
            w1_sb = [
                wpool.tile([128, 2, HID], f8, tag=f"w1_{kp}", name=f"w1_{kp}")
                for kp in range(KP1)
            ]
            wh_sb = wpool.tile([128, NHL, HP, 2, HT, 128], f8, tag="wh")
            w3_sb = wpool.tile([128, HP, 2, L], f8, tag="w3")
            v_sb = wpool.tile([128, NVEC], f32, tag="vecs")

            # The PE never idles in steady state: the four L1 matmul chains
            # of chunk c+1 (1.7us of act-independent work each) are emitted
            # inside chunk c's hidden/out phases so every end-of-layer
            # activation barrier is covered by L1' work:
            #   [hl0 kp0|kp1] L1' ht0 [hl1 kp0|kp1] L1' ht1+ht2
            #   [out kp0|kp1] L1' ht3
            def act(out_sl, ps_sl, scol, bcol, eng, l1=False):
                # out = relu(scale*ps + bias); bias==0 on the fast V path
                if eng == "V" and zero_bias:
                    sc = 1.0 / WS if l1 else v_sb[:, scol : scol + 1]
                    nc.vector.tensor_scalar(out_sl, ps_sl, sc, 0.0, mult, maxop)
                else:
                    sc = COL_SL1 if l1 else scol
                    nc.scalar.activation(
                        out_sl,
                        ps_sl,
                        Relu,
                        bias=v_sb[:, bcol : bcol + 1],
                        scale=v_sb[:, sc : sc + 1],
                    )

            def l1_chain(ht, x_t, ps):
                for kp in range(KP1):
                    nc.tensor.matmul(
                        ps[:],
                        w1_sb[kp][:, :, ht * 128 : (ht + 1) * 128],
                        x_t[:, kp],
                        start=(kp == 0),
                        stop=(kp == KP1 - 1),
                        perf_mode=DR,
                    )

            L1_ENG = ("S", "V", "S", "SV")  # ht3 half-split across both

            def l1_piece(ht, x_t, h1n):
                ps = ps1pool.tile([128, NCH], f32, tag="ps1", name=f"psl1_{ht}")
                l1_chain(ht, x_t, ps)
                dst = h1n[ht // 2]
                eng = L1_ENG[ht]
                if eng == "SV" and zero_bias:
                    # the last chain ends at the period boundary; split its
                    # act across both engines so it lands before the next
                    # chunk's hidden kp1 group needs it
                    half = NCH // 2
                    act(dst[:, ht % 2, :half], ps[:, :half], None, COL_B1 + ht, "S",
                        l1=True)
                    act(dst[:, ht % 2, half:], ps[:, half:], None, COL_B1 + ht, "V",
                        l1=True)
                else:
                    act(dst[:, ht % 2, :], ps[:], None, COL_B1 + ht,
                        "S" if eng == "SV" else eng, l1=True)

            def hidden_layer(l, hin):
                hout = [
                    hpool.tile(
                        [128, 2, NCH], f8, tag=f"h{l + 2}_{p}", name=f"h{l + 2}_{p}"
                    )
                    for p in range(HP)
                ]
                pss = [
                    pshpool.tile([128, NCH], f32, tag=f"ps2_{mt}", name=f"ps2_{mt}")
                    for mt in range(HT)
                ]
                for mt in range(HT):
                    nc.tensor.matmul(
                        pss[mt][:],
                        wh_sb[:, l, 0, :, mt, :],
                        hin[0][:],
                        start=True,
                        stop=False,
                        perf_mode=DR,
                    )
                for mt in range(HT):
                    nc.tensor.matmul(
                        pss[mt][:],
                        wh_sb[:, l, 1, :, mt, :],
                        hin[1][:],
                        start=False,
                        stop=True,
                        perf_mode=DR,
                    )
                    act(
                        hout[mt // 2][:, mt % 2, :],
                        pss[mt][:],
                        COL_S(l) + mt,
                        COL_C(l) + mt,
                        "S" if mt % 2 == 0 else "V",
                    )
                return hout

            # ---- prologue: weights + x0 (sliced) + x1, chunk-0 L1 ----
            x_c = xpool.tile([128, KP1, 2, NCH], f8, tag="x", name="x_0")
            for kp in range(KP1):
                nc.sync.dma_start(out=w1_sb[kp][:], in_=w1[:, kp])
                nc.sync.dma_start(out=x_c[:, kp], in_=xT[:, 0, kp])
            z_c = zpool.tile([128, LT, NCH], f32, tag="z", name="z_0")
            nc.sync.dma_start(out=z_c[:], in_=zT[:, 0])
            nc.sync.dma_start(out=wh_sb[:], in_=wh[:])
            nc.sync.dma_start(out=w3_sb[:], in_=w3[:])
            nc.sync.dma_start(out=v_sb[:], in_=vecs[:])
            x_n = None
            if nchunk > 1:
                x_n = xpool.tile([128, KP1, 2, NCH], f8, tag="x", name="x_1")
                nc.sync.dma_start(out=x_n[:], in_=xT[:, 1])
            h1c = [
                hpool.tile([128, 2, NCH], f8, tag=f"h1_{p}", name=f"h1_{p}")
                for p in range(HP)
            ]
            for ht in range(HT):
                l1_piece_ps = ps1pool.tile([128, NCH], f32, tag="ps1", name="ps0")
                l1_chain(ht, x_c, l1_piece_ps)
                act(
                    h1c[ht // 2][:, ht % 2, :],
                    l1_piece_ps[:],
                    None,
                    COL_B1 + ht,
                    "S" if ht % 2 == 0 else "V",
                    l1=True,
                )

            for c in range(nchunk):
                last = c == nchunk - 1
                # prefetch x two chunks ahead: its DMA trigger is processed
                # ~one full period before chunk c+1's L1' chains read it
                if c + 2 < nchunk:
                    x_f = xpool.tile([128, KP1, 2, NCH], f8, tag="x", name="x_f")
                    nc.sync.dma_start(out=x_f[:], in_=xT[:, c + 2])
                z_n = None
                h1n = None
                if not last:
                    z_n = zpool.tile([128, LT, NCH], f32, tag="z", name="z_n")
                    nc.sync.dma_start(out=z_n[:], in_=zT[:, c + 1])
                    h1n = [
                        hpool.tile([128, 2, NCH], f8, tag=f"h1_{p}", name=f"h1n_{p}")
                        for p in range(HP)
                    ]

                h2 = hidden_layer(0, h1c)
                if not last:
                    l1_piece(0, x_n, h1n)
                h3 = hidden_layer(1, h2)
                if not last:
                    l1_piece(1, x_n, h1n)
                    l1_piece(2, x_n, h1n)

                # ---- out: delta^T = W3^T h3; out = delta^T + (zlast+b3)^T
                # out psums recycle the hidden-layer banks (ps2_0..2), so the
                # ps1 ring stays dedicated to the four L1' chains
                pso = [
                    pshpool.tile([128, NCH], f32, tag=f"ps2_{lt}", name=f"pso_{lt}")
                    for lt in range(LT)
                ]
                for lt in range(LT):
                    nc.tensor.matmul(
                        pso[lt][:],
                        w3_sb[:, 0, :, lt * 128 : (lt + 1) * 128],
                        h3[0][:],
                        start=True,
                        stop=False,
                        perf_mode=DR,
                    )
                for lt in range(LT):
                    nc.tensor.matmul(
                        pso[lt][:],
                        w3_sb[:, 1, :, lt * 128 : (lt + 1) * 128],
                        h3[1][:],
                        start=False,
                        stop=True,
                        perf_mode=DR,
                    )
                    ot = opool.tile([128, NCH], f32, tag=f"o{lt}", name=f"o{lt}")
                    nc.vector.tensor_tensor(ot[:], pso[lt][:], z_c[:, lt, :], add)
                    nc.sync.dma_start(
                        out=outT[lt * 128 : (lt + 1) * 128, c * NCH : (c + 1) * NCH],
                        in_=ot[:],
                    )
                if not last:
                    l1_piece(3, x_n, h1n)
                    h1c, z_c = h1n, z_n
                    x_n = x_f if c + 2 < nchunk else None
    nc.compile()
    return nc


def _f8():
    import ml_dtypes

    return ml_dtypes.float8_e4m3


def prep_core_inputs(
    z_hist, a_hist, W1, b1, Wh, bh, gamma, beta, rmean, rvar, W3, b3
):
    """Host-side shard prep: returns per-model input dicts (xT shared)."""
    f8 = _f8()
    batch = z_hist.shape[0]
    nchunk = batch // NCH
    x = np.concatenate(
        [z_hist.reshape(batch, -1), a_hist.reshape(batch, -1)], axis=1
    ).astype(np.float32)
    xpad = np.zeros((batch, DINP), np.float32)
    xpad[:, :DIN] = x
    xq = xpad.astype(f8)  # quantize once, then pure byte shuffles
    xT8 = np.ascontiguousarray(
        xq.reshape(nchunk, NCH, DINP // 128, 128).transpose(3, 0, 2, 1)
    ).reshape(128, nchunk, KP1, 2, NCH)

    z_last = z_hist[:, -1, :].astype(np.float32)  # [batch, L]

    rstd = 1.0 / np.sqrt(rvar.astype(np.float64) + EPS)  # [NHL, M, HID]
    s_aff = (gamma * rstd).astype(np.float32)
    c_aff = ((bh - rmean) * gamma * rstd + beta).astype(np.float32)

    in_maps = []
    for m in range(M):
        w1p = np.zeros((DINP, HID), np.float32)
        w1p[:DIN] = W1[m] * WS
        w1h = np.ascontiguousarray(
            w1p.astype(f8).reshape(DINP // 128, 128, HID).transpose(1, 0, 2)
        ).reshape(128, KP1, 2, HID)

        whh = np.ascontiguousarray(
            (Wh[:, m] * WS)
            .astype(np.float32)
            .astype(f8)
            .reshape(NHL, HT, 128, HT, 128)
            .transpose(2, 0, 1, 3, 4)
        ).reshape(128, NHL, HP, 2, HT, 128)

        w3h = np.ascontiguousarray(
            W3[m].astype(np.float32).astype(f8).reshape(HT, 128, L).transpose(1, 0, 2)
        ).reshape(128, HP, 2, L)

        vecs = np.zeros((128, NVEC), np.float32)
        vecs[:, COL_B1 : COL_B1 + HT] = b1[m].reshape(HT, 128).T
        for l in range(NHL):
            vecs[:, COL_S(l) : COL_S(l) + HT] = (s_aff[l, m] / WS).reshape(HT, 128).T
            vecs[:, COL_C(l) : COL_C(l) + HT] = c_aff[l, m].reshape(HT, 128).T
        vecs[:, COL_SL1] = 1.0 / WS

        zb = z_last + b3[m][None, :]  # fold b3 into the residual stream
        zTm = np.ascontiguousarray(
            zb.reshape(nchunk, NCH, LT, 128).transpose(3, 0, 2, 1)
        )  # [128, nchunk, LT, NCH]
        in_maps.append(
            {"xT": xT8, "zT": zTm, "w1": w1h, "wh": whh, "w3": w3h, "vecs": vecs}
        )
    return in_maps


def _reset_device():
    """Clear any exec-unit wedge a previous (profiled) session left behind."""
    try:
        import ctypes

        import jax

        jax.devices()
        lib = ctypes.CDLL("/opt/axon/libaxon_pjrt.so")
        if hasattr(lib, "axon_reset"):
            lib.axon_reset.restype = ctypes.c_int64
            lib.axon_reset()
    except Exception:
        pass


def is_zero_bias(inputs):
    """True iff every additive term of the per-layer affines is zero, i.e.
    the activations reduce to relu(scale * psum)."""
    return not (
        inputs["b1"].any()
        or inputs["bh"].any()
        or inputs["beta"].any()
        or inputs["rmean"].any()
    )


def kernel(**inputs):
    inputs = {k: np.asarray(v) for k, v in inputs.items()}
    in_maps = prep_core_inputs(**inputs)
    nc = build_bass(B, zero_bias=is_zero_bias(inputs))

    from concourse import bass_utils

    _reset_device()
    res = bass_utils.run_bass_kernel_spmd(nc, in_maps, core_ids=list(range(M)))
    out = np.stack(
        [np.ascontiguousarray(res.results[m]["outT"].T) for m in range(M)]
    )  # [M, B, L]
    return out.astype(np.float32)


# revision 17
# speedup vs baseline: 1.7938x; 1.0026x over previous
"""Trainium2 Bass kernel for nn_EnsembleTransitionModel.

Sharding: model-parallel (expert-parallel). M=8 ensemble members across 8
NeuronCores; each core runs one full MLP over the whole batch. Inputs are
replicated, per-model weights are sharded.

All four matmul layers run in fp8 (e4m3) DoubleRow perf mode: each matmul
instruction contracts K=256 (two 128-k-tiles packed as pairs in the free
dim of both operands) at 2 MACs/cell/cycle — 2x bf16 throughput. fp32
accumulation in PSUM.

Precision plan (measured rel-err ~2.2e-3 vs the 2e-2 gate):
  - W1, Wh are scaled x64 before fp8 quantization (raw weights ~N(0,0.02)
    sit in e4m3's subnormal range); the 1/64 rides the existing per-feature
    affine applied by the scalar-engine Relu activation out of PSUM.
  - W3 stays unscaled (error is the same either way) so the output stage is
    a single fp32 vector add of the residual stream, exactly as in bf16.
  - The residual z_last (+ b3 folded) rides a separate fp32 path so the
    dominant output term stays full precision.
  - DIN=1925 is zero-padded to 2048 so the 5 a_hist rows fold into the main
    L1 matmul (no separate host-computed rank-5 term).

Layouts: activations feature-major (x^T: [features, batch]); x is packed
chunk-major in DRAM ([128, chunk, kpair, 2, 512]) so each chunk's DMA is
one fully-contiguous 8KB-per-partition transfer.
"""

import os
import sys

import numpy as np

for _p in ("/opt/trn_rl_repo", "/root/.axon_site/_ro/trn_rl_repo"):
    if os.path.isdir(_p) and _p not in sys.path:
        sys.path.insert(0, _p)

M = 8
B = 16384
HIST = 5
L = 384
A = 1
HID = 512
NHL = 2
DIN = L * HIST + A * HIST  # 1925
EPS = 1e-5

WS = 64.0  # fp8 weight pre-scale for W1/Wh (compensated in the affine)

NCH = 512  # batch columns per chunk (= max fp32 moving dim = 1 PSUM bank)
DINP = 2048  # DIN zero-padded to 16 k-tiles
KP1 = DINP // 256  # 8 L1 k-pairs (DoubleRow consumes 2 k-tiles per matmul)
HT = HID // 128  # 4 hidden feature tiles
HP = HT // 2  # 2 hidden k-pairs
LT = L // 128  # 3 output feature tiles
ZROW0 = (HIST - 1) * L  # 1536: first row of z_last within x^T

# vecs columns: [b1 (4) | s0 (4) | c0 (4) | s1 (4) | c1 (4) | sL1 (1)]
COL_B1 = 0
COL_S = lambda l: 4 + 8 * l
COL_C = lambda l: 8 + 8 * l
COL_SL1 = 4 + 8 * NHL
NVEC = COL_SL1 + 1


def build_bass(batch=B, zero_bias=True):
    """zero_bias=True (true for this model instance: b1/bh/beta/rmean all
    zero) routes half the activations to the Vector engine as a one-pass
    relu(scale*psum) tensor_scalar, halving the end-of-layer activation
    barrier the PE waits on. With nonzero biases everything stays on the
    Scalar engine's general affine activation path."""
    import concourse.bacc as bacc
    import concourse.tile as tile
    from concourse import mybir

    f32 = mybir.dt.float32
    f8 = mybir.dt.float8e4
    DR = mybir.MatmulPerfMode.DoubleRow
    Relu = mybir.ActivationFunctionType.Relu
    add = mybir.AluOpType.add
    mult = mybir.AluOpType.mult
    maxop = mybir.AluOpType.max

    nchunk = batch // NCH
    assert nchunk * NCH == batch

    nc = bacc.Bacc("TRN2", target_bir_lowering=False)
    xT = nc.declare_dram_parameter("xT", [128, nchunk, KP1, 2, NCH], f8, isOutput=False)
    zT = nc.declare_dram_parameter("zT", [128, nchunk, LT, NCH], f32, isOutput=False)
    w1 = nc.declare_dram_parameter("w1", [128, KP1, 2, HID], f8, isOutput=False)
    wh = nc.declare_dram_parameter("wh", [128, NHL, HP, 2, HT, 128], f8, isOutput=False)
    w3 = nc.declare_dram_parameter("w3", [128, HP, 2, L], f8, isOutput=False)
    vecs = nc.declare_dram_parameter("vecs", [128, NVEC], f32, isOutput=False)
    outT = nc.declare_dram_parameter("outT", [L, batch], f32, isOutput=True)

    with tile.TileContext(nc) as tc:
        with (
            tc.tile_pool(name="wt", bufs=1) as wpool,
            tc.tile_pool(name="x", bufs=3) as xpool,
            tc.tile_pool(name="z", bufs=2) as zpool,
            tc.tile_pool(name="h", bufs=2) as hpool,
            tc.tile_pool(name="o", bufs=3) as opool,
            tc.tile_pool(name="ps1", bufs=4, space="PSUM") as ps1pool,
            tc.tile_pool(name="psh", bufs=1, space="PSUM") as pshpool,
        ):
            # per-k-pair W1 tiles so the first matmul only waits on its own
            # 128KB slice, not the whole 1MB preload (DMAs are interleaved
            # with chunk-0 x slices, kp by kp)
            w1_sb = [
                wpool.tile([128, 2, HID], f8, tag=f"w1_{kp}", name=f"w1_{kp}")
                for kp in range(KP1)
            ]
            wh_sb = wpool.tile([128, NHL, HP, 2, HT, 128], f8, tag="wh")
            w3_sb = wpool.tile([128, HP, 2, L], f8, tag="w3")
            v_sb = wpool.tile([128, NVEC], f32, tag="vecs")

            # The PE never idles in steady state: the four L1 matmul chains
            # of chunk c+1 (1.7us of act-independent work each) are emitted
            # inside chunk c's hidden/out phases so every end-of-layer
            # activation barrier is covered by L1' work:
            #   [hl0 kp0|kp1] L1' ht0 [hl1 kp0|kp1] L1' ht1+ht2
            #   [out kp0|kp1] L1' ht3
            def act(out_sl, ps_sl, scol, bcol, eng, l1=False):
                # out = relu(scale*ps + bias); bias==0 on the fast V path
                if eng == "V" and zero_bias:
                    sc = 1.0 / WS if l1 else v_sb[:, scol : scol + 1]
                    nc.vector.tensor_scalar(out_sl, ps_sl, sc, 0.0, mult, maxop)
                else:
                    sc = COL_SL1 if l1 else scol
                    nc.scalar.activation(
                        out_sl,
                        ps_sl,
                        Relu,
                        bias=v_sb[:, bcol : bcol + 1],
                        scale=v_sb[:, sc : sc + 1],
                    )

            def l1_chain(ht, x_t, ps):
                for kp in range(KP1):
                    nc.tensor.matmul(
                        ps[:],
                        w1_sb[kp][:, :, ht * 128 : (ht + 1) * 128],
                        x_t[:, kp],
                        start=(kp == 0),
                        stop=(kp == KP1 - 1),
                        perf_mode=DR,
                    )

            L1_ENG = ("S", "V", "S", "SV")  # ht3 half-split across both

            def l1_piece(ht, x_t, h1n):
                ps = ps1pool.tile([128, NCH], f32, tag="ps1", name=f"psl1_{ht}")
                l1_chain(ht, x_t, ps)
                dst = h1n[ht // 2]
                eng = L1_ENG[ht]
                if eng == "SV" and zero_bias:
                    # the last chain ends at the period boundary; split its
                    # act across both engines so it lands before the next
                    # chunk's hidden kp1 group needs it
                    half = NCH // 2
                    act(dst[:, ht % 2, :half], ps[:, :half], None, COL_B1 + ht, "S",
                        l1=True)
                    act(dst[:, ht % 2, half:], ps[:, half:], None, COL_B1 + ht, "V",
                        l1=True)
                else:
                    act(dst[:, ht % 2, :], ps[:], None, COL_B1 + ht,
                        "S" if eng == "SV" else eng, l1=True)

            def hidden_layer(l, hin):
                hout = [
                    hpool.tile(
                        [128, 2, NCH], f8, tag=f"h{l + 2}_{p}", name=f"h{l + 2}_{p}"
                    )
                    for p in range(HP)
                ]
                pss = [
                    pshpool.tile([128, NCH], f32, tag=f"ps2_{mt}", name=f"ps2_{mt}")
                    for mt in range(HT)
                ]
                for mt in range(HT):
                    nc.tensor.matmul(
                        pss[mt][:],
                        wh_sb[:, l, 0, :, mt, :],
                        hin[0][:],
                        start=True,
                        stop=False,
                        perf_mode=DR,
                    )
                for mt in range(HT):
                    nc.tensor.matmul(
                        pss[mt][:],
                        wh_sb[:, l, 1, :, mt, :],
                        hin[1][:],
                        start=False,
                        stop=True,
                        perf_mode=DR,
                    )
                    dst = hout[mt // 2]
                    if zero_bias:
                        # halve the act latency: Scalar and Vector each
                        # process half the columns concurrently, so the
                        # act barriers the PE waits on shrink by ~300ns
                        half = NCH // 2
                        act(
                            dst[:, mt % 2, :half],
                            pss[mt][:, :half],
                            COL_S(l) + mt,
                            COL_C(l) + mt,
                            "S",
                        )
                        act(
                            dst[:, mt % 2, half:],
                            pss[mt][:, half:],
                            COL_S(l) + mt,
                            COL_C(l) + mt,
                            "V",
                        )
                    else:
                        act(
                            dst[:, mt % 2, :],
                            pss[mt][:],
                            COL_S(l) + mt,
                            COL_C(l) + mt,
                            "S",
                        )
                return hout

            # ---- prologue: weights + x0 (sliced) + x1, chunk-0 L1 ----
            x_c = xpool.tile([128, KP1, 2, NCH], f8, tag="x", name="x_0")
            for kp in range(KP1):
                nc.sync.dma_start(out=w1_sb[kp][:], in_=w1[:, kp])
                nc.sync.dma_start(out=x_c[:, kp], in_=xT[:, 0, kp])
            z_c = zpool.tile([128, LT, NCH], f32, tag="z", name="z_0")
            nc.sync.dma_start(out=z_c[:], in_=zT[:, 0])
            nc.sync.dma_start(out=wh_sb[:], in_=wh[:])
            nc.sync.dma_start(out=w3_sb[:], in_=w3[:])
            nc.sync.dma_start(out=v_sb[:], in_=vecs[:])
            x_n = None
            if nchunk > 1:
                x_n = xpool.tile([128, KP1, 2, NCH], f8, tag="x", name="x_1")
                nc.sync.dma_start(out=x_n[:], in_=xT[:, 1])
            h1c = [
                hpool.tile([128, 2, NCH], f8, tag=f"h1_{p}", name=f"h1_{p}")
                for p in range(HP)
            ]
            for ht in range(HT):
                l1_piece_ps = ps1pool.tile([128, NCH], f32, tag="ps1", name="ps0")
                l1_chain(ht, x_c, l1_piece_ps)
                act(
                    h1c[ht // 2][:, ht % 2, :],
                    l1_piece_ps[:],
                    None,
                    COL_B1 + ht,
                    "S" if ht % 2 == 0 else "V",
                    l1=True,
                )

            for c in range(nchunk):
                last = c == nchunk - 1
                # prefetch x two chunks ahead: its DMA trigger is processed
                # ~one full period before chunk c+1's L1' chains read it
                if c + 2 < nchunk:
                    x_f = xpool.tile([128, KP1, 2, NCH], f8, tag="x", name="x_f")
                    nc.sync.dma_start(out=x_f[:], in_=xT[:, c + 2])
                z_n = None
                h1n = None
                if not last:
                    z_n = zpool.tile([128, LT, NCH], f32, tag="z", name="z_n")
                    nc.sync.dma_start(out=z_n[:], in_=zT[:, c + 1])
                    h1n = [
                        hpool.tile([128, 2, NCH], f8, tag=f"h1_{p}", name=f"h1n_{p}")
                        for p in range(HP)
                    ]

                h2 = hidden_layer(0, h1c)
                if not last:
                    l1_piece(0, x_n, h1n)
                h3 = hidden_layer(1, h2)
                if not last:
                    l1_piece(1, x_n, h1n)
                    l1_piece(2, x_n, h1n)

                # ---- out: delta^T = W3^T h3; out = delta^T + (zlast+b3)^T
                # out psums recycle the hidden-layer banks (ps2_0..2), so the
                # ps1 ring stays dedicated to the four L1' chains
                pso = [
                    pshpool.tile([128, NCH], f32, tag=f"ps2_{lt}", name=f"pso_{lt}")
                    for lt in range(LT)
                ]
                for lt in range(LT):
                    nc.tensor.matmul(
                        pso[lt][:],
                        w3_sb[:, 0, :, lt * 128 : (lt + 1) * 128],
                        h3[0][:],
                        start=True,
                        stop=False,
                        perf_mode=DR,
                    )
                for lt in range(LT):
                    nc.tensor.matmul(
                        pso[lt][:],
                        w3_sb[:, 1, :, lt * 128 : (lt + 1) * 128],
                        h3[1][:],
                        start=False,
                        stop=True,
                        perf_mode=DR,
                    )
                    ot = opool.tile([128, NCH], f32, tag=f"o{lt}", name=f"o{lt}")
                    nc.vector.tensor_tensor(ot[:], pso[lt][:], z_c[:, lt, :], add)
                    nc.sync.dma_start(
                        out=outT[lt * 128 : (lt + 1) * 128, c * NCH : (c + 1) * NCH],
                        in_=ot[:],
                    )
                if not last:
                    l1_piece(3, x_n, h1n)
                    h1c, z_c = h1n, z_n
                    x_n = x_f if c + 2 < nchunk else None
    nc.compile()
    return nc


def _f8():
    import ml_dtypes

    return ml_dtypes.float8_e4m3


def prep_core_inputs(
    z_hist, a_hist, W1, b1, Wh, bh, gamma, beta, rmean, rvar, W3, b3
):
    """Host-side shard prep: returns per-model input dicts (xT shared)."""
    f8 = _f8()
    batch = z_hist.shape[0]
    nchunk = batch // NCH
    x = np.concatenate(
        [z_hist.reshape(batch, -1), a_hist.reshape(batch, -1)], axis=1
    ).astype(np.float32)
    xpad = np.zeros((batch, DINP), np.float32)
    xpad[:, :DIN] = x
    xq = xpad.astype(f8)  # quantize once, then pure byte shuffles
    xT8 = np.ascontiguousarray(
        xq.reshape(nchunk, NCH, DINP // 128, 128).transpose(3, 0, 2, 1)
    ).reshape(128, nchunk, KP1, 2, NCH)

    z_last = z_hist[:, -1, :].astype(np.float32)  # [batch, L]

    rstd = 1.0 / np.sqrt(rvar.astype(np.float64) + EPS)  # [NHL, M, HID]
    s_aff = (gamma * rstd).astype(np.float32)
    c_aff = ((bh - rmean) * gamma * rstd + beta).astype(np.float32)

    in_maps = []
    for m in range(M):
        w1p = np.zeros((DINP, HID), np.float32)
        w1p[:DIN] = W1[m] * WS
        w1h = np.ascontiguousarray(
            w1p.astype(f8).reshape(DINP // 128, 128, HID).transpose(1, 0, 2)
        ).reshape(128, KP1, 2, HID)

        whh = np.ascontiguousarray(
            (Wh[:, m] * WS)
            .astype(np.float32)
            .astype(f8)
            .reshape(NHL, HT, 128, HT, 128)
            .transpose(2, 0, 1, 3, 4)
        ).reshape(128, NHL, HP, 2, HT, 128)

        w3h = np.ascontiguousarray(
            W3[m].astype(np.float32).astype(f8).reshape(HT, 128, L).transpose(1, 0, 2)
        ).reshape(128, HP, 2, L)

        vecs = np.zeros((128, NVEC), np.float32)
        vecs[:, COL_B1 : COL_B1 + HT] = b1[m].reshape(HT, 128).T
        for l in range(NHL):
            vecs[:, COL_S(l) : COL_S(l) + HT] = (s_aff[l, m] / WS).reshape(HT, 128).T
            vecs[:, COL_C(l) : COL_C(l) + HT] = c_aff[l, m].reshape(HT, 128).T
        vecs[:, COL_SL1] = 1.0 / WS

        zb = z_last + b3[m][None, :]  # fold b3 into the residual stream
        zTm = np.ascontiguousarray(
            zb.reshape(nchunk, NCH, LT, 128).transpose(3, 0, 2, 1)
        )  # [128, nchunk, LT, NCH]
        in_maps.append(
            {"xT": xT8, "zT": zTm, "w1": w1h, "wh": whh, "w3": w3h, "vecs": vecs}
        )
    return in_maps


def _reset_device():
    """Clear any exec-unit wedge a previous (profiled) session left behind."""
    try:
        import ctypes

        import jax

        jax.devices()
        lib = ctypes.CDLL("/opt/axon/libaxon_pjrt.so")
        if hasattr(lib, "axon_reset"):
            lib.axon_reset.restype = ctypes.c_int64
            lib.axon_reset()
    except Exception:
        pass


def is_zero_bias(inputs):
    """True iff every additive term of the per-layer affines is zero, i.e.
    the activations reduce to relu(scale * psum)."""
    return not (
        inputs["b1"].any()
        or inputs["bh"].any()
        or inputs["beta"].any()
        or inputs["rmean"].any()
    )


def kernel(**inputs):
    inputs = {k: np.asarray(v) for k, v in inputs.items()}
    in_maps = prep_core_inputs(**inputs)
    nc = build_bass(B, zero_bias=is_zero_bias(inputs))

    from concourse import bass_utils

    _reset_device()
    res = bass_utils.run_bass_kernel_spmd(nc, in_maps, core_ids=list(range(M)))
    out = np.stack(
        [np.ascontiguousarray(res.results[m]["outT"].T) for m in range(M)]
    )  # [M, B, L]
    return out.astype(np.float32)
